# revision 36
# baseline (speedup 1.0000x reference)
"""AdaptiveStdPooling2d on 8 TRN2 NeuronCores.

Input  x: [16, 128, 512, 128] f32.  Output: [16, 128, 8, 16] f32.
out[b,c,i,j] = sum_{kw=0..7} std_h(x[b, c, 64*i:64*i+64, 8*j+kw])
with biased variance over the 64-row bin plus EPS=1e-14 inside sqrt.

Sharding: pure data parallel over batch B=16 -> 2 per core, no collectives.

Per-core kernel: C=128 channels on SBUF partitions.  For each (b, bin_h)
DMA a [128c, 64h, 128w] tile (contiguous 32 KiB per partition), square it
on the scalar engine, segmented reduce_sum over the 64-row bins on the
vector engine for both x and x^2, var = E[x^2]-E[x]^2, sqrt on the scalar
engine, reduce_sum over kw=8 into the output tile.
"""

import numpy as np

B, C, H, W = 16, 128, 512, 128
N_CORES = 8
B_LOC = B // N_CORES          # 2 batches per core
H_OUT, W_OUT = 8, 16
KH, KW = H // H_OUT, W // W_OUT   # 64, 8
EPS = 1e-14

_CACHE = {}


def _build_bass(reps=1, variant="full"):
    import concourse.bacc as bacc
    import concourse.mybir as mybir
    from concourse import tile

    f32 = mybir.dt.float32
    nc = bacc.Bacc(None, target_bir_lowering=False)
    x_in = nc.declare_dram_parameter("x", [B_LOC, C, H, W], f32, isOutput=False)
    out = nc.declare_dram_parameter("out", [B_LOC, C, H_OUT, W_OUT], f32, isOutput=True)

    with tile.TileContext(nc) as tc:
        with (
            tc.tile_pool(name="xp", bufs=3) as xp,
            tc.tile_pool(name="sq", bufs=2) as sqp,
            tc.tile_pool(name="tp", bufs=3) as tp,
            tc.tile_pool(name="op", bufs=1) as op,
        ):
            oacc = op.tile([C, B_LOC, H_OUT, W_OUT], f32, tag="oacc")
            eps_t = op.tile([C, 1], f32, tag="eps")
            nc.vector.memset(eps_t[:], float(EPS))
            if variant == "dma":
                nc.vector.memset(oacc[:], 0.0)
            for b, ih in [(b, ih) for _ in range(reps)
                          for b in range(B_LOC) for ih in range(H_OUT)]:
                    xt = xp.tile([C, KH, W], f32, tag="x")
                    nc.sync.dma_start(out=xt[:], in_=x_in[b, :, ih * KH:(ih + 1) * KH, :])
                    if variant == "dma":
                        continue
                    s1 = tp.tile([C, W], f32, tag="s1")
                    s2 = tp.tile([C, W], f32, tag="s2")
                    if variant in ("full", "nosq"):
                        if variant == "full":
                            sq = sqp.tile([C, KH, W], f32, tag="sq")
                            nc.scalar.activation(
                                sq[:], xt[:], mybir.ActivationFunctionType.Square,
                            )
                        else:
                            sq = xt
                        nc.vector.reduce_sum(
                            out=s2[:], in_=sq[:].transpose([0, 2, 1]),
                            axis=mybir.AxisListType.X,
                        )
                    # reduce over the h axis (innermost after transpose)
                    nc.vector.reduce_sum(
                        out=s1[:], in_=xt[:].transpose([0, 2, 1]),
                        axis=mybir.AxisListType.X,
                    )
                    if variant == "onepass":
                        s2 = s1
                    # var = s2/64 - (s1/64)^2
                    mean = tp.tile([C, W], f32, tag="mean")
                    m2 = tp.tile([C, W], f32, tag="m2")
                    var = tp.tile([C, W], f32, tag="var")
                    stds = tp.tile([C, W], f32, tag="stds")
                    nc.vector.tensor_scalar_mul(mean[:], s1[:], 1.0 / KH)
                    nc.vector.tensor_mul(m2[:], mean[:], mean[:])
                    nc.vector.scalar_tensor_tensor(
                        out=var[:], in0=s2[:], scalar=1.0 / KH, in1=m2[:],
                        op0=mybir.AluOpType.mult, op1=mybir.AluOpType.subtract,
                    )
                    nc.scalar.activation(
                        stds[:], var[:], mybir.ActivationFunctionType.Sqrt,
                        bias=eps_t[:], scale=1.0,
                    )
                    nc.vector.reduce_sum(
                        out=oacc[:, b, ih, :],
                        in_=stds[:].rearrange("p (g k) -> p g k", k=KW),
                        axis=mybir.AxisListType.X,
                    )
            nc.sync.dma_start(out=out.transpose([1, 0, 2, 3]), in_=oacc[:])
    nc.finalize()
    return nc


def _build_pe(reps=1, variant="full", n_dve=0, sq_alt=False, gp_dve=False):
    """Pure TensorEngine reduction path.

    Layout: h (within a 128-row chunk = 2 bins) on partitions.  Per chunk
    (b, hc, cc) of [128h, 64c, 128w]:
      DVE casts x -> bf16, ACT squares x -> bf16, PE reduces both over the
      two 64-row bins via ldweights(X)/matmul(selector) pairs (out [128w, 2]
      per channel), DVE combines into var (fp32), ACT sqrt, and a second
      tiny PE matmul against a kw-selector sums std over w-groups of 8,
      landing [c, w_out] with c back on partitions.
    """
    import concourse.bacc as bacc
    import concourse.mybir as mybir
    from concourse import tile

    f32 = mybir.dt.float32
    bf16 = mybir.dt.bfloat16
    Alu = mybir.AluOpType
    CCH = 64                      # channels per chunk
    CC = C // CCH                 # chunks per (b, hc)
    HCN = H // 128                # 4 h-chunks (2 bins each)

    nc = bacc.Bacc(None, target_bir_lowering=False)
    x_in = nc.declare_dram_parameter("x", [B_LOC, C, H, W], f32, isOutput=False)
    out = nc.declare_dram_parameter("out", [B_LOC, C, H_OUT, W_OUT], f32, isOutput=True)

    with tile.TileContext(nc) as tc:
        with (
            tc.tile_pool(name="pbp", bufs=3 if n_dve else 6) as pbp,
            tc.tile_pool(name="pqp", bufs=2 if n_dve else 4) as pqp,
            tc.tile_pool(name="dxp", bufs=2) as dxp,
            tc.tile_pool(name="dqp", bufs=1) as dqp,
            tc.tile_pool(name="ptp", bufs=3) as ptp,
            tc.tile_pool(name="psx", bufs=2, space="PSUM") as psx,
            tc.tile_pool(name="pso", bufs=2, space="PSUM") as pso,
            tc.tile_pool(name="op", bufs=1) as op,
        ):
            oacc = op.tile([C, B_LOC, H_OUT, W_OUT], f32, tag="oacc")
            eps_t = op.tile([C, 1], f32, tag="eps")
            nc.vector.memset(eps_t[:], float(EPS))
            # selector constants
            sel2f = op.tile([128, 2], f32, tag="sel2f")
            nc.vector.memset(sel2f[:], 1.0)
            # keep iff 0 <= p - 64*j <= 63  (i.e. j == p // 64)
            nc.gpsimd.affine_select(
                out=sel2f[:], in_=sel2f[:], pattern=[[-KH, 2]],
                compare_op=Alu.is_ge, fill=0.0, base=0, channel_multiplier=1,
            )
            nc.gpsimd.affine_select(
                out=sel2f[:], in_=sel2f[:], pattern=[[KH, 2]],
                compare_op=Alu.is_ge, fill=0.0, base=KH - 1, channel_multiplier=-1,
            )
            sel2b = op.tile([128, 2], bf16, tag="sel2b")
            nc.vector.tensor_copy(sel2b[:], sel2f[:])
            kwsel = op.tile([128, W_OUT], f32, tag="kwsel")
            nc.vector.memset(kwsel[:], 1.0)
            # keep iff 0 <= p - 8*j <= 7  (i.e. j == p // 8)
            nc.gpsimd.affine_select(
                out=kwsel[:], in_=kwsel[:], pattern=[[-KW, W_OUT]],
                compare_op=Alu.is_ge, fill=0.0, base=0, channel_multiplier=1,
            )
            nc.gpsimd.affine_select(
                out=kwsel[:], in_=kwsel[:], pattern=[[KW, W_OUT]],
                compare_op=Alu.is_ge, fill=0.0, base=KW - 1, channel_multiplier=-1,
            )

            dve_sel = {3: (1, 4, 6), 2: (1, 5), 1: (3,), 0: (),
                       4: (1, 3, 4, 6)}[n_dve]
            for _ in range(reps):
                for b in range(B_LOC):
                    for hc in range(HCN):
                        if b * HCN + hc in dve_sel and variant == "full":
                            # fast p=c loads + DVE segmented reduces
                            for ih in (2 * hc, 2 * hc + 1):
                                xt = dxp.tile([C, KH, W], f32, tag="xtf")
                                nc.sync.dma_start(
                                    out=xt[:],
                                    in_=x_in[b, :, ih * KH:(ih + 1) * KH, :],
                                )
                                sqf = dqp.tile([C, KH, W], f32, tag="sqf")
                                nc.scalar.activation(
                                    sqf[:], xt[:],
                                    mybir.ActivationFunctionType.Square,
                                )
                                s1 = ptp.tile([C, W], f32, tag="s1")
                                s2 = ptp.tile([C, W], f32, tag="s2")
                                if gp_dve:
                                    # sum-x on GpSimd: in-place log-fold on xt
                                    # (safe: Square already consumed xt)
                                    hh = KH // 2
                                    while hh >= 1:
                                        dst = s1[:] if hh == 1 else xt[:, 0:hh, :]
                                        nc.gpsimd.tensor_add(
                                            dst, xt[:, 0:hh, :] if hh > 1 else xt[:, 0, :],
                                            xt[:, hh:2 * hh, :] if hh > 1 else xt[:, 1, :],
                                        )
                                        hh //= 2
                                else:
                                    nc.vector.reduce_sum(
                                        out=s1[:], in_=xt[:].transpose([0, 2, 1]),
                                        axis=mybir.AxisListType.X,
                                    )
                                nc.vector.reduce_sum(
                                    out=s2[:], in_=sqf[:].transpose([0, 2, 1]),
                                    axis=mybir.AxisListType.X,
                                )
                                meanv = ptp.tile([C, W], f32, tag="meanv")
                                m2v = ptp.tile([C, W], f32, tag="m2v")
                                varv = ptp.tile([C, W], f32, tag="varv")
                                stds = ptp.tile([C, W], f32, tag="stds")
                                nc.vector.tensor_scalar_mul(meanv[:], s1[:], 1.0 / KH)
                                nc.vector.tensor_mul(m2v[:], meanv[:], meanv[:])
                                nc.vector.scalar_tensor_tensor(
                                    out=varv[:], in0=s2[:], scalar=1.0 / KH,
                                    in1=m2v[:], op0=Alu.mult, op1=Alu.subtract,
                                )
                                nc.scalar.activation(
                                    stds[:], varv[:],
                                    mybir.ActivationFunctionType.Sqrt,
                                    bias=eps_t[:], scale=1.0,
                                )
                                nc.vector.reduce_sum(
                                    out=oacc[:, b, ih, :],
                                    in_=stds[:].rearrange("p (g k) -> p g k", k=KW),
                                    axis=mybir.AxisListType.X,
                                )
                            continue
                        ps_o = pso.tile([128, 2, W_OUT], f32, tag="ps_o")
                        # one cast-during-DMA load of the whole [128h, C, W]
                        # unit (8 MiB HBM-side, 4 MiB bf16 SBUF-side, SWDGE)
                        if variant == "dmah":
                            # HWDGE fp32 loads, same transposed 512B-run pattern
                            pxf = pbp.tile([128, C // 2, W], f32, tag="pxf")
                            for dh in range(2):
                                nc.sync.dma_start(
                                    out=pxf[:],
                                    in_=x_in[
                                        b, dh * (C // 2):(dh + 1) * (C // 2),
                                        hc * 128:(hc + 1) * 128, :,
                                    ].transpose([1, 0, 2]),
                                )
                            continue
                        for cc in range(CC):
                            pxb = pbp.tile([128, CCH, W], bf16, tag="pxb")
                            nc.gpsimd.dma_start(
                                out=pxb[:],
                                in_=x_in[
                                    b, cc * CCH:(cc + 1) * CCH,
                                    hc * 128:(hc + 1) * 128, :,
                                ].transpose([1, 0, 2]),
                            )
                            if variant == "dma":
                                continue
                            psqb = pqp.tile([128, CCH, W], bf16, tag="psqb")
                            if sq_alt and cc % 2 == 0:
                                nc.vector.tensor_mul(psqb[:], pxb[:], pxb[:])
                            else:
                                nc.scalar.activation(
                                    psqb[:], pxb[:],
                                    mybir.ActivationFunctionType.Square,
                                )
                            ps_x = psx.tile([128, 2 * CCH], f32, tag="ps_x")
                            ps_q = psx.tile([128, 2 * CCH], f32, tag="ps_q")
                            for c0 in range(CCH):
                                nc.tensor.matmul(
                                    ps_x[:, 2 * c0:2 * c0 + 2],
                                    pxb[:, c0, :], sel2b[:],
                                    start=True, stop=True,
                                )
                                nc.tensor.matmul(
                                    ps_q[:, 2 * c0:2 * c0 + 2],
                                    psqb[:, c0, :], sel2b[:],
                                    start=True, stop=True,
                                )
                            mean = ptp.tile([128, 2 * CCH], f32, tag="mean")
                            m2 = ptp.tile([128, 2 * CCH], f32, tag="m2")
                            var = ptp.tile([128, 2 * CCH], f32, tag="var")
                            pstd = ptp.tile([128, 2, CCH], f32, tag="pstd")
                            nc.vector.tensor_scalar_mul(mean[:], ps_x[:], 1.0 / KH)
                            nc.vector.tensor_mul(m2[:], mean[:], mean[:])
                            nc.vector.scalar_tensor_tensor(
                                out=var[:], in0=ps_q[:], scalar=1.0 / KH,
                                in1=m2[:], op0=Alu.mult, op1=Alu.subtract,
                            )
                            nc.scalar.activation(
                                pstd[:].transpose([0, 2, 1]),
                                var[:].rearrange("p (c t) -> p c t", t=2),
                                mybir.ActivationFunctionType.Sqrt,
                                bias=eps_t[:], scale=1.0,
                            )
                            for bin_ in range(2):
                                nc.tensor.matmul(
                                    ps_o[cc * CCH:(cc + 1) * CCH, bin_, :],
                                    pstd[:, bin_, :], kwsel[:],
                                    start=True, stop=True,
                                )
                        if variant == "dma":
                            continue
                        nc.vector.tensor_copy(
                            oacc[:, b, 2 * hc:2 * hc + 2, :], ps_o[:],
                        )
            if variant in ("dma", "dmah"):
                nc.vector.memset(oacc[:], 0.0)
            nc.sync.dma_start(out=out.transpose([1, 0, 2, 3]), in_=oacc[:])
    nc.finalize()
    return nc


def _build_fold(reps=1, variant="full", n_dve_sq=1, gp_fold=True):
    """p=c layout everywhere (fast 32KiB-contiguous loads, bf16 cast in DMA).

    Per (b, ih) tile [C, KH=64, W] bf16: square on ACT (a few tiles' squares
    go to DVE), then segmented sum over the 64-row bin via log2 folding
    (tensor_add): the first fold level (half the work) runs on GpSimd, the
    rest on DVE at bf16 2x.  Final level accumulates to fp32.  var/std/kw-sum
    as usual.
    """
    import concourse.bacc as bacc
    import concourse.mybir as mybir
    from concourse import tile

    f32 = mybir.dt.float32
    bf16 = mybir.dt.bfloat16
    Alu = mybir.AluOpType

    nc = bacc.Bacc(None, target_bir_lowering=False)
    x_in = nc.declare_dram_parameter("x", [B_LOC, C, H, W], f32, isOutput=False)
    out = nc.declare_dram_parameter("out", [B_LOC, C, H_OUT, W_OUT], f32, isOutput=True)

    with tile.TileContext(nc) as tc:
        with (
            tc.tile_pool(name="xbp", bufs=3) as xbp,
            tc.tile_pool(name="sqp", bufs=2) as sqp,
            tc.tile_pool(name="fp", bufs=2) as fp,
            tc.tile_pool(name="tp", bufs=3) as tp,
            tc.tile_pool(name="op", bufs=1) as op,
        ):
            oacc = op.tile([C, B_LOC, H_OUT, W_OUT], f32, tag="oacc")
            eps_t = op.tile([C, 1], f32, tag="eps")
            nc.vector.memset(eps_t[:], float(EPS))

            def fold_sum(src, out_s1, tag):
                # src [C, 64, W] bf16 -> out_s1 [C, W] f32 (sum over axis 1).
                # First level on GpSimd, mid levels bf16@2x on DVE, tail fp32.
                h = KH // 2
                eng = nc.gpsimd if gp_fold else nc.vector
                cur = fp.tile([C, h, W], bf16, tag=f"{tag}{h}")
                eng.tensor_add(cur[:], src[:, 0:h, :], src[:, h:2 * h, :])
                while h > 8:
                    h //= 2
                    nxt = fp.tile([C, h, W], bf16, tag=f"{tag}{h}")
                    nc.vector.tensor_add(nxt[:], cur[:, 0:h, :], cur[:, h:2 * h, :])
                    cur = nxt
                while h > 2:
                    h //= 2
                    nxt = fp.tile([C, h, W], f32, tag=f"{tag}f{h}")
                    nc.vector.tensor_add(nxt[:], cur[:, 0:h, :], cur[:, h:2 * h, :])
                    cur = nxt
                nc.vector.tensor_add(out_s1[:], cur[:, 0, :], cur[:, 1, :])

            ti = 0
            for _ in range(reps):
                for b in range(B_LOC):
                    for ih in range(H_OUT):
                        xb = xbp.tile([C, KH, W], bf16, tag="xb")
                        nc.gpsimd.dma_start(
                            out=xb[:], in_=x_in[b, :, ih * KH:(ih + 1) * KH, :],
                        )
                        if variant == "dma":
                            continue
                        sqb = sqp.tile([C, KH, W], bf16, tag="sqb")
                        if ti % H_OUT < n_dve_sq:
                            nc.vector.tensor_mul(sqb[:], xb[:], xb[:])
                        else:
                            nc.scalar.activation(
                                sqb[:], xb[:], mybir.ActivationFunctionType.Square,
                            )
                        ti += 1
                        s1 = tp.tile([C, W], f32, tag="s1")
                        s2 = tp.tile([C, W], f32, tag="s2")
                        fold_sum(xb, s1, "fx")
                        fold_sum(sqb, s2, "fq")
                        meanv = tp.tile([C, W], f32, tag="meanv")
                        m2v = tp.tile([C, W], f32, tag="m2v")
                        varv = tp.tile([C, W], f32, tag="varv")
                        stds = tp.tile([C, W], f32, tag="stds")
                        nc.vector.tensor_scalar_mul(meanv[:], s1[:], 1.0 / KH)
                        nc.vector.tensor_mul(m2v[:], meanv[:], meanv[:])
                        nc.vector.scalar_tensor_tensor(
                            out=varv[:], in0=s2[:], scalar=1.0 / KH,
                            in1=m2v[:], op0=Alu.mult, op1=Alu.subtract,
                        )
                        nc.scalar.activation(
                            stds[:], varv[:], mybir.ActivationFunctionType.Sqrt,
                            bias=eps_t[:], scale=1.0,
                        )
                        nc.vector.reduce_sum(
                            out=oacc[:, b, ih, :],
                            in_=stds[:].rearrange("p (g k) -> p g k", k=KW),
                            axis=mybir.AxisListType.X,
                        )
            if variant == "dma":
                nc.vector.memset(oacc[:], 0.0)
            nc.sync.dma_start(out=out.transpose([1, 0, 2, 3]), in_=oacc[:])
    nc.finalize()
    return nc


def kernel(x):
    import os
    from concourse.bass_utils import run_bass_kernel_spmd

    x = np.ascontiguousarray(np.asarray(x, dtype=np.float32))
    assert x.shape == (B, C, H, W), x.shape

    impl = os.environ.get("KERNEL_IMPL", "pe3")
    if _CACHE.get("impl") != impl:
        if impl.startswith("pe"):
            rest = impl[2:]
            sq_alt = rest.endswith("s")
            if sq_alt:
                rest = rest[:-1]
            _CACHE["nc"] = _build_pe(n_dve=int(rest) if rest else 0, sq_alt=sq_alt)
        elif impl.startswith("fold"):
            _CACHE["nc"] = _build_fold()
        else:
            _CACHE["nc"] = _build_bass()
        _CACHE["impl"] = impl
    nc = _CACHE["nc"]

    in_maps = [{"x": x[i * B_LOC:(i + 1) * B_LOC]} for i in range(N_CORES)]
    last_err = None
    for _ in range(3):
        try:
            res = run_bass_kernel_spmd(nc, in_maps, core_ids=list(range(N_CORES)))
            break
        except Exception as e:  # transient axon/device hiccups
            last_err = e
    else:
        raise last_err
    return np.concatenate([np.asarray(r["out"]) for r in res.results], axis=0)
